# revision 15
# baseline (speedup 1.0000x reference)
"""Trainium2 Bass kernel for the Koopman-operator rollout.

Reference computation: y0 = x[:, 0, :]  (shape [2048, 256]);
    y_t = y_{t-1} @ W.T  for t = 1..512, Y[:, t-1, :] = y_t.
Output: [2048, 512, 256] fp32 (1 GiB) -> memory-bound target.

Strategy (8 cores, data-parallel over batch, 256 rows/core):
  Let Wt = W.T.  Y[:, t] = y0 @ Wt^{t+1}.
  * P_j = Wt^j for j=1..16 built as two product chains (evens
    P_{j+2} = P_j Wt^2, odds similarly) so consecutive P's become
    available at product-pipeline rate, plus a single helper Q2 = W^2.
    Products use matmul(out = lhsT.T @ rhs) with natural layouts.
  * Checkpoint states Z_i = y0 @ Wt^{16 i} advance SEQUENTIALLY:
    Z_{i+1}^T = A1^T... i.e. Z_{i+1}^T = (Wt^16 applied) via
    _product(zt_next, P16, zt_prev); emissions run in checkpoint order
    so Z availability always leads the DMA stream by many checkpoints.
  * Per checkpoint i: Y[:, 16i+j-1] = Z_i @ P_j for j=1..16, as dense
    N=512 matmuls with Z_i^T stationary; PSUM -> SBUF copies on
    DVE/ACT; 2 MiB HWDGE DMAs to HBM.
  * Checkpoint 0 is emitted pair-by-pair interleaved with the P-chain
    construction so the output DMA stream starts as soon as P2 exists
    (~7 us) instead of after the full tree (~27 us).  A burst of dummy
    PE matmuls at t=0 ramps the tensor engine to full clock before the
    real prologue work arrives.
  Matmul-operand tiles are float32r (full PE rate at N>=256, fp32 PSUM
  accumulation); PSUM->SBUF copies perform the f32->f32r rounding.

  Cost-model timeline: ~381 us/core vs a ~374 us HBM-write floor for
  the 128 MiB/core output.
"""

import os

import numpy as np

import concourse.bass as bass
import concourse.mybir as mybir
import concourse.tile as tile
from concourse import bacc
from concourse.bass import ds
from concourse.bass_utils import run_bass_kernel_spmd
from concourse.masks import make_identity

F32 = mybir.dt.float32
F32R = mybir.dt.float32r

N_CORES = 8
B_FULL = 2048
B_SH = B_FULL // N_CORES  # 256 batch rows per core
K = 256  # state dim
T = 512  # time steps
S = 16  # timesteps per checkpoint chunk
M = T // S  # 32 checkpoints

# engine choice for PSUM->SBUF output copies: every Nth tile on ScalarE
ACT_COPY_EVERY = int(os.environ.get("K_ACT_EVERY", "2"))
N_DUMMY = int(os.environ.get("K_DUMMY", "6"))


def _mm(nc, out, lhsT, rhs, start, stop):
    # operands are float32r tiles already (producers round to f32r)
    nc.tensor.matmul(out, lhsT, rhs, start=start, stop=stop)


class _Mat:
    """A 256x256 matrix stored as an SBUF tile [128, 2, 256]:
    elem (p, h, c) = M[h*128 + p, c]."""

    def __init__(self, ap):
        self.ap = ap

    def half(self, hm):
        # [128, 256] slice: rows hm*128 .. hm*128+127 (partition = row)
        return self.ap[:, hm, :]

    def blk(self, hm, hc):
        # [128, 128] block: rows hm*128.., cols hc*128..
        return self.ap[:, hm, ds(128 * hc, 128)]

    def whole(self):
        # [128, 2, 256]: both row-halves (copy destination)
        return self.ap


_prod_ctr = [0]


def _product(nc, psum_pool, dst, lhsT_mat, rhs_mat, copy_eng=None):
    """dst = lhsT_mat.T @ rhs_mat  (all 256x256 _Mats).

    Both column-halves share one PSUM bank ([128, 2, 256]) and drain with a
    single copy (engine selectable), so a product costs 1 PSUM slot,
    4 matmuls, 1 copy."""
    _prod_ctr[0] += 1
    ps = psum_pool.tile([128, 2, 256], F32, tag="psz", name=f"psz_{_prod_ctr[0]}")
    for ha in range(2):
        for hm in range(2):
            _mm(nc, ps[:, ha, :], lhsT_mat.blk(hm, ha), rhs_mat.half(hm), hm == 0, hm == 1)
    _copy(nc, copy_eng or "v", dst.whole(), ps)


def _copy(nc, eng, dst, src):
    if eng == "v":
        nc.vector.tensor_copy(dst, src)
    else:
        nc.scalar.copy(dst, src)


def _build_program():
    nc = bacc.Bacc(
        "TRN2",
        target_bir_lowering=False,
        debug=False,
        enable_asserts=False,
        num_devices=N_CORES,
    )
    x_d = nc.dram_tensor("x", [B_SH, K], F32, kind="ExternalInput").ap()
    w_d = nc.dram_tensor("w", [K, K], F32, kind="ExternalInput").ap()
    y_d = nc.dram_tensor("y", [B_SH, T, K], F32, kind="ExternalOutput").ap()

    with tile.TileContext(nc) as tc:
        with (
            tc.tile_pool(name="consts", bufs=1) as consts,
            tc.tile_pool(name="mats", bufs=1) as mats,
            tc.tile_pool(name="zts", bufs=1) as zts,
            tc.tile_pool(name="ostage", bufs=int(os.environ.get("K_OST", "3"))) as ostage,
            tc.tile_pool(name="pso", bufs=int(os.environ.get("K_PSO", "4")), space="PSUM") as pso,
            tc.tile_pool(name="psz", bufs=int(os.environ.get("K_PSZ", "4")), space="PSUM") as psz,
        ):
            ident = consts.tile([128, 128], F32, tag="ident", name="ident")
            make_identity(nc, ident)

            # --- PE clock warm-up: back-to-back dummy matmuls from t~0 so
            # the tensor engine is at full p-state when real work lands.
            for d in range(N_DUMMY):
                warm = psz.tile([128, 128], F32, tag="psz", name=f"warm_{d}")
                _mm(nc, warm, ident, ident, True, True)

            # --- inputs: W first (gates the P-chain), x in parallel on ACT
            w_nat = consts.tile([128, 2, K], F32, tag="w_nat", name="w_nat")
            x_nat = consts.tile([128, 2, K], F32, tag="x_nat", name="x_nat")
            w_v = w_d.rearrange("(h p) k -> p h k", p=128)
            x_v = x_d.rearrange("(h p) k -> p h k", p=128)
            nc.sync.dma_start(out=w_nat, in_=w_v)
            nc.scalar.dma_start(out=x_nat, in_=x_v)

            # Pcat holds P_1..P_16 row-half-major: [128, 2, 16*256]
            pcat = mats.tile([128, 2, S * K], F32R, tag="pcat", name="pcat")

            def P(j):  # 1-indexed power as a _Mat-like view
                class V:
                    def half(self, hm, _j=j):
                        return pcat[:, hm, ds(K * (_j - 1), K)]

                    def blk(self, hm, hc, _j=j):
                        return pcat[:, hm, ds(K * (_j - 1) + 128 * hc, 128)]

                    def whole(self, _j=j):
                        return pcat[:, :, ds(K * (_j - 1), K)]

                return V()

            w_r = consts.tile([128, 2, K], F32R, tag="w_r", name="w_r")
            for h in range(2):
                nc.vector.tensor_copy(w_r[:, h, :], w_nat[:, h, :])
            q1 = _Mat(w_r)  # Q_1 = W (natural layout, rounded to f32r)

            # --- transposes: P_1 = W^T first (gates everything), then
            # Z0^T = x^T (x arrives later; its DMA overlaps W's use on PE)
            zt_pool_bufs = 3
            zt0 = _Mat(zts.tile([128, 2, K], F32R, tag="zt", bufs=zt_pool_bufs, name="zt0"))
            p1 = P(1)
            for g in range(2):
                for h in range(2):
                    pst2 = psz.tile([128, 128], F32, tag="psz", name=f"pstw_{g}_{h}")
                    nc.tensor.transpose(pst2, w_nat[:, g, ds(128 * h, 128)], ident)
                    _copy(nc, "v" if h == 0 else "s", pcat[:, h, ds(128 * g, 128)], pst2)
            for g in range(2):
                for h in range(2):
                    pst = psz.tile([128, 128], F32, tag="psz", name=f"pstx_{g}_{h}")
                    nc.tensor.transpose(pst, x_nat[:, g, ds(128 * h, 128)], ident)
                    _copy(nc, "v" if h == 0 else "s", zt0.ap[:, h, ds(128 * g, 128)], pst)

            copy_ctr = [0]

            def pos_copy(dst, src, force_vector=False):
                if not force_vector and (
                    copy_ctr[0] % ACT_COPY_EVERY == ACT_COPY_EVERY - 1
                ):
                    nc.scalar.copy(dst, src)
                else:
                    nc.vector.tensor_copy(dst, src)
                copy_ctr[0] += 1

            y_r = y_d.rearrange("(h p) t k -> p h t k", p=128)

            def emit_c0(j0, w, split=False):
                """Y[:, j0-1 : j0-1+w] = Z0 @ [P_{j0} .. P_{j0+w-1}] (j0 is
                1-indexed).  Both batch halves stage into one
                [128, 2, w, K] tile; `split` issues one DMA per half (used
                for the first unit, earliest possible stream start), else
                one merged DMA on SP."""
                ost = ostage.tile(
                    [128, 2, w, K], F32, tag=f"ost0w{w}", bufs=4, name=f"ost0_{j0}"
                )
                for m in range(2):
                    pos = pso.tile([128, w, K], F32, tag="pso", name=f"pso0_{m}_{j0}")
                    for hm in range(2):
                        lhsT = zt0.ap[:, hm, ds(128 * m, 128)]
                        rhs = pcat[:, hm, ds(K * (j0 - 1), K * w)]
                        _mm(nc, pos, lhsT, rhs, hm == 0, hm == 1)
                    _copy(nc, "v" if m == 0 else "s", ost[:, m, :, :], pos)
                    if split:
                        dma_eng = nc.sync if m == 0 else nc.scalar
                        dma_eng.dma_start(
                            out=y_d[ds(128 * m, 128), ds(j0 - 1, w), :],
                            in_=ost[:, m, :, :],
                        )
                if not split:
                    nc.sync.dma_start(
                        out=y_r[:, :, ds(j0 - 1, w), :], in_=ost
                    )

            # --- P-chain interleaved with checkpoint-0 emission, software
            # pipelined one unit deep: each emission unit is queued on PE
            # after the products of the NEXT unit, so emission matmuls never
            # park in PE's in-order queue waiting for a product copy.
            # evens chain: P_{j+2} = Wt^2 P_j (lhsT = Q2); odds likewise.
            # Emission units: single j=1 (no products needed), pairs
            # (2,3), (4,5), ..., (14,15), single j=16.
            q2 = _Mat(mats.tile([128, 2, K], F32R, tag="q2", name="q2"))
            _product(nc, psz, P(2), q1, p1, "v")  # Wt^2
            _product(nc, psz, P(3), q1, P(2), "s")  # Wt^3
            emit_c0(1, 1, split=True)
            _product(nc, psz, q2, p1, q1, "v")  # Q2 = W^2
            _product(nc, psz, P(4), q2, P(2), "s")  # Wt^4
            _product(nc, psz, P(5), q2, P(3), "v")  # Wt^5
            emit_c0(2, 2)
            for j in range(6, S, 2):
                _product(nc, psz, P(j), q2, P(j - 2), "s")  # even chain
                _product(nc, psz, P(j + 1), q2, P(j - 1), "v")  # odd chain
                emit_c0(j - 2, 2)
            _product(nc, psz, P(S), q2, P(S - 2), "s")  # Wt^16
            emit_c0(S - 2, 2)

            def emit_outputs(i, zt_i):
                """Y[:, 16i + j - 1, :] = Z_i @ P_j for j=1..16, staged as
                one 16-step SBUF tile + one 2 MiB DMA per batch half."""
                for m in range(2):  # batch half
                    ost = ostage.tile(
                        [128, S, K], F32, tag="ost", name=f"ost_{i}_{m}"
                    )
                    pos = {}
                    for n in range(8):
                        pos[n] = pso.tile(
                            [128, 2, K], F32, tag="pso", name=f"pso_{i}_{m}_{n}"
                        )
                    for hm in range(2):
                        lhsT = zt_i.ap[:, hm, ds(128 * m, 128)]
                        for n in range(8):
                            rhs = pcat[:, hm, ds(512 * n, 512)]
                            _mm(nc, pos[n], lhsT, rhs, hm == 0, hm == 1)
                    for n in range(8):
                        pos_copy(ost[:, ds(2 * n, 2), :], pos[n])
                    nc.sync.dma_start(
                        out=y_d[ds(128 * m, 128), ds(S * i, S), :],
                        in_=ost,
                    )

            # --- sequential checkpoint rollout: Z_{i+1}^T = A1^T @ Z_i^T
            # (A1 = P16 = Wt^16); emission order = checkpoint order, so Z
            # availability always leads the DMA stream.
            a1 = P(S)
            zt_prev = zt0
            for i in range(1, M):
                zt_next = _Mat(
                    zts.tile([128, 2, K], F32R, tag="zt", bufs=zt_pool_bufs, name=f"zt{i}")
                )
                _product(nc, psz, zt_next, a1, zt_prev)
                emit_outputs(i, zt_next)
                zt_prev = zt_next

    nc.compile()
    return nc


_cached_nc = None
_last_results = None


def kernel(x, W, T=None):
    global _cached_nc, _last_results
    if _cached_nc is None:
        _cached_nc = _build_program()
    nc = _cached_nc

    x2 = np.ascontiguousarray(np.asarray(x, dtype=np.float32).reshape(B_FULL, K))
    w2 = np.ascontiguousarray(np.asarray(W, dtype=np.float32))
    in_maps = [
        {"x": x2[i * B_SH : (i + 1) * B_SH], "w": w2} for i in range(N_CORES)
    ]
    res = run_bass_kernel_spmd(
        nc,
        in_maps,
        core_ids=list(range(N_CORES)),
        trace=bool(os.environ.get("BASS_TRACE")),
    )
    _last_results = res
    y = np.concatenate([res.results[i]["y"] for i in range(N_CORES)], axis=0)
    return y


# revision 66
# speedup vs baseline: 1.0340x; 1.0340x over previous
"""Trainium2 Bass kernel for the Koopman-operator rollout.

Reference computation: y0 = x[:, 0, :]  (shape [2048, 256]);
    y_t = y_{t-1} @ W.T  for t = 1..512, Y[:, t-1, :] = y_t.
Output: [2048, 512, 256] fp32 (1 GiB) -> memory-bound target; the whole
kernel is paced by the ~374 us it takes to write 128 MiB/core to HBM at
360 GB/s, so the design goal is a DMA stream with no idle slots.

Strategy (8 cores, data-parallel over batch, 256 rows/core):
  Let Wt = W.T.  Y[:, t] = y0 @ Wt^{t+1}.
  * P_j = Wt^j for j=1..16 held in one SBUF strip (pcat).  P2/P3 come
    off W^T directly; P4 off a short sequential hop; P5..P16 via THREE
    concurrent chains P_j = (W^3)^T P_{j-3} (helper Q3), so fresh
    powers land every ~0.45 us during the prologue instead of the
    ~1.3 us serial product cadence.
  * Offset-8 checkpointing: checkpoint 0 covers only t=0..7
    (Z1 = Z0 Wt^8 needs just P8); checkpoint i>=1 covers
    t = 8+16(i-1)..+15.  The P-chain construction then amortizes over
    24 prologue-emitted timesteps, keeping PE matmul work per DMA slot
    under the DMA cadence, and there is no staged-checkpoint seam at
    the end of c0.  Z_{i+1}^T = (Wt^16)^T Z_i^T advances sequentially
    (error growth checked against a 10-bit-mantissa numpy emulation:
    same as the prefix-doubling ladder, which emission-operand
    rounding dominates).
  * Prologue emissions are latency-shaped: four 1-step units (j=1 as
    soon as only P1/Z0 exist), then 2-step units gated pair-by-pair on
    the P-chains, staged via one PSUM bank + DVE/ACT drain copies and
    SP-issued DMAs.  Steady state: one 16-step staged tile + one 2 MiB
    DMA per batch half per checkpoint (supply runs ~2x ahead).
  * Warm-up: dummy PE matmuls from t~0 hold the tensor engine at full
    p-state before real work lands; a 1-element ACT copy hoists the
    1.3 us activation-table load off the critical path; W loads before
    x (the product chain hanging off W is longer).
  Matmul-operand tiles are float32r (full PE rate at N>=256, fp32 PSUM
  accumulation); PSUM->SBUF copies perform the f32->f32r rounding.

  Cost-model timeline: ~386 us/core vs the ~374 us HBM-write floor
  (startup DMA lead-in ~2 us + prologue fill ~8 us + tail sem ~1.6 us).
"""

import os

import numpy as np

import concourse.bass as bass
import concourse.mybir as mybir
import concourse.tile as tile
from concourse import bacc
from concourse.bass import ds
from concourse.bass_utils import run_bass_kernel_spmd
from concourse.masks import make_identity

F32 = mybir.dt.float32
F32R = mybir.dt.float32r

N_CORES = 8
B_FULL = 2048
B_SH = B_FULL // N_CORES  # 256 batch rows per core
K = 256  # state dim
T = 512  # time steps
S = 16  # timesteps per checkpoint chunk
M = T // S  # 32 checkpoints

# engine choice for PSUM->SBUF output copies: every Nth tile on ScalarE
ACT_COPY_EVERY = int(os.environ.get("K_ACT_EVERY", "2"))
N_DUMMY = int(os.environ.get("K_DUMMY", "6"))


def _mm(nc, out, lhsT, rhs, start, stop):
    # operands are float32r tiles already (producers round to f32r)
    nc.tensor.matmul(out, lhsT, rhs, start=start, stop=stop)


class _Mat:
    """A 256x256 matrix stored as an SBUF tile [128, 2, 256]:
    elem (p, h, c) = M[h*128 + p, c]."""

    def __init__(self, ap):
        self.ap = ap

    def half(self, hm):
        # [128, 256] slice: rows hm*128 .. hm*128+127 (partition = row)
        return self.ap[:, hm, :]

    def blk(self, hm, hc):
        # [128, 128] block: rows hm*128.., cols hc*128..
        return self.ap[:, hm, ds(128 * hc, 128)]

    def whole(self):
        # [128, 2, 256]: both row-halves (copy destination)
        return self.ap


_prod_ctr = [0]


def _product(nc, psum_pool, dst, lhsT_mat, rhs_mat, copy_eng=None):
    """dst = lhsT_mat.T @ rhs_mat  (all 256x256 _Mats).

    Both column-halves share one PSUM bank ([128, 2, 256]) and drain with a
    single copy (engine selectable), so a product costs 1 PSUM slot,
    4 matmuls, 1 copy."""
    _prod_ctr[0] += 1
    ps = psum_pool.tile([128, 2, 256], F32, tag="psz", name=f"psz_{_prod_ctr[0]}")
    for ha in range(2):
        for hm in range(2):
            _mm(nc, ps[:, ha, :], lhsT_mat.blk(hm, ha), rhs_mat.half(hm), hm == 0, hm == 1)
    _copy(nc, copy_eng or "v", dst.whole(), ps)


def _copy(nc, eng, dst, src):
    # NOTE: "g" (GPSIMD/Pool) cannot read PSUM on real HW -- only use it
    # for SBUF->SBUF copies.
    if eng == "v":
        nc.vector.tensor_copy(dst, src)
    elif eng == "g":
        nc.gpsimd.tensor_copy(dst, src)
    else:
        nc.scalar.copy(dst, src)


def _build_program():
    nc = bacc.Bacc(
        "TRN2",
        target_bir_lowering=False,
        debug=False,
        enable_asserts=False,
        num_devices=N_CORES,
    )
    x_d = nc.dram_tensor("x", [B_SH, K], F32, kind="ExternalInput").ap()
    w_d = nc.dram_tensor("w", [K, K], F32, kind="ExternalInput").ap()
    y_d = nc.dram_tensor("y", [B_SH, T, K], F32, kind="ExternalOutput").ap()

    with tile.TileContext(nc) as tc:
        with (
            tc.tile_pool(name="consts", bufs=1) as consts,
            tc.tile_pool(name="mats", bufs=1) as mats,
            tc.tile_pool(name="zts", bufs=1) as zts,
            tc.tile_pool(name="ostage", bufs=int(os.environ.get("K_OST", "3"))) as ostage,
            tc.tile_pool(name="pso", bufs=int(os.environ.get("K_PSO", "4")), space="PSUM") as pso,
            tc.tile_pool(name="psz", bufs=int(os.environ.get("K_PSZ", "4")), space="PSUM") as psz,
        ):
            ident = consts.tile([128, 128], F32, tag="ident", name="ident")
            make_identity(nc, ident)

            # --- PE clock warm-up: back-to-back dummy matmuls from t~0 so
            # the tensor engine is at full p-state when real work lands.
            for d in range(N_DUMMY):
                warm = psz.tile([128, 128], F32, tag="psz", name=f"warm_{d}")
                _mm(nc, warm, ident, ident, True, True)

            # --- prime the Activation engine's function table at t~0 so the
            # 1.3us LoadActFuncSet doesn't land in front of the first real
            # ACT copy on the critical path.
            actwarm = consts.tile([128, 1], F32, tag="actwarm", name="actwarm")
            nc.scalar.copy(actwarm, ident[:, 0:1])

            # --- inputs: W first on SP (the product chain hanging off W is
            # longer than x's transpose chain), x in parallel on ACT
            w_nat = consts.tile([128, 2, K], F32, tag="w_nat", name="w_nat")
            x_nat = consts.tile([128, 2, K], F32, tag="x_nat", name="x_nat")
            w_v = w_d.rearrange("(h p) k -> p h k", p=128)
            x_v = x_d.rearrange("(h p) k -> p h k", p=128)
            nc.sync.dma_start(out=w_nat, in_=w_v)
            nc.scalar.dma_start(out=x_nat, in_=x_v)

            # Pcat holds P_1..P_16 row-half-major: [128, 2, 16*256]
            pcat = mats.tile([128, 2, S * K], F32R, tag="pcat", name="pcat")

            def P(j):  # 1-indexed power as a _Mat-like view
                class V:
                    def half(self, hm, _j=j):
                        return pcat[:, hm, ds(K * (_j - 1), K)]

                    def blk(self, hm, hc, _j=j):
                        return pcat[:, hm, ds(K * (_j - 1) + 128 * hc, 128)]

                    def whole(self, _j=j):
                        return pcat[:, :, ds(K * (_j - 1), K)]

                return V()

            # --- transposes: P_1 = W^T first (W lands first), then
            # Z0^T = x^T; psum->sbuf copies split across DVE/ACT
            zt_pool_bufs = 3
            zt0 = _Mat(zts.tile([128, 2, K], F32R, tag="zt", bufs=zt_pool_bufs, name="zt0"))
            p1 = P(1)
            # both g-blocks of a half share one PSUM tile -> single drain copy
            for h in range(2):
                pst2 = psz.tile([128, 2, 128], F32, tag="psz", name=f"pstw_{h}")
                for g in range(2):
                    nc.tensor.transpose(
                        pst2[:, g, :], w_nat[:, g, ds(128 * h, 128)], ident
                    )
                _copy(nc, "v" if h == 0 else "s", pcat[:, h, ds(0, K)], pst2)
            for h in range(2):
                pst = psz.tile([128, 2, 128], F32, tag="psz", name=f"pstx_{h}")
                for g in range(2):
                    nc.tensor.transpose(
                        pst[:, g, :], x_nat[:, g, ds(128 * h, 128)], ident
                    )
                _copy(nc, "v" if h == 0 else "s", zt0.ap[:, h, :], pst)
            # q1 operand (rounded W, natural layout) on the otherwise-idle
            # Pool engine so it gates the first product as early as possible
            w_r = consts.tile([128, 2, K], F32R, tag="w_r", name="w_r")
            _copy(nc, "g", w_r, w_nat)
            q1 = _Mat(w_r)

            copy_ctr = [0]

            def pos_copy(dst, src, force_vector=False):
                if not force_vector and (
                    copy_ctr[0] % ACT_COPY_EVERY == ACT_COPY_EVERY - 1
                ):
                    nc.scalar.copy(dst, src)
                else:
                    nc.vector.tensor_copy(dst, src)
                copy_ctr[0] += 1

            y_r = y_d.rearrange("(h p) t k -> p h t k", p=128)

            pair_ctr = [0]

            def emit_pair(zt_i, jg, t0):
                """Y[:, t0 : t0+2] = Z_i @ [P_{jg} | P_{jg+1}] (jg is
                1-indexed).  Per-batch-half PSUM tiles, staged into one
                [128, 2, 2, K] tile (copies split DVE/ACT), one merged DMA
                on SP.  Used for the prologue stream (c0 + c1 front) where
                per-pair gating on the P-chain matters."""
                pair_ctr[0] += 1
                ost = ostage.tile(
                    [128, 2, 2, K], F32, tag="ost0", bufs=4, name=f"ostp_{t0}"
                )
                for m in range(2):
                    pos = pso.tile([128, 2, K], F32, tag="pso", name=f"posp_{m}_{t0}")
                    for hm in range(2):
                        lhsT = zt_i.ap[:, hm, ds(128 * m, 128)]
                        rhs = pcat[:, hm, ds(K * (jg - 1), 2 * K)]
                        _mm(nc, pos, lhsT, rhs, hm == 0, hm == 1)
                    _copy(nc, "v" if m == 0 else "s", ost[:, m, :, :], pos)
                nc.sync.dma_start(out=y_r[:, :, ds(t0, 2), :], in_=ost)

            def emit_outputs(zt_i, t0, j0, w):
                """Y[:, t0 : t0+w] = Z_i @ [P_{j0} .. P_{j0+w-1}], staged as
                one w-step SBUF tile + one DMA per batch half (steady
                state; supply runs far ahead of the DMA stream here)."""
                for m in range(2):  # batch half
                    ost = ostage.tile(
                        [128, w, K], F32, tag=f"ost{w}", name=f"ost_{t0}_{m}"
                    )
                    pos = {}
                    for n in range(w // 2):
                        pos[n] = pso.tile(
                            [128, 2, K], F32, tag="pso", name=f"pso_{t0}_{m}_{n}"
                        )
                    for hm in range(2):
                        lhsT = zt_i.ap[:, hm, ds(128 * m, 128)]
                        for n in pos:
                            rhs = pcat[:, hm, ds(K * (j0 - 1) + 512 * n, 512)]
                            _mm(nc, pos[n], lhsT, rhs, hm == 0, hm == 1)
                    for n in pos:
                        pos_copy(ost[:, ds(2 * n, 2), :], pos[n])
                    nc.sync.dma_start(
                        out=y_d[ds(128 * m, 128), ds(t0, w), :],
                        in_=ost,
                    )

            # --- Offset-8 checkpointing: checkpoint 0 covers only t=0..7
            # (Z1 = Z0 Wt^8 needs just P8), checkpoint i>=1 covers
            # t = 8+16(i-1) .. +15.  The P-chain construction (16 products)
            # then amortizes over 24 pair-granular emitted timesteps, which
            # brings PE work per 2-step DMA slot under the DMA cadence --
            # the stream never starves once it starts, and there is no
            # staged-checkpoint seam at the end of c0.
            # evens chain: P_{j+2} = Wt^2 P_j (lhsT = Q2); odds likewise.
            def emit_single(zt_i, j, t0, ceng, deng):
                """Y[:, t0] = Z_i @ P_j: both batch halves in one PSUM bank,
                one staging copy, one small DMA.  Prologue front only."""
                pos = pso.tile([128, 2, 1, K], F32, tag="pso", name=f"poss_{t0}")
                for m in range(2):
                    for hm in range(2):
                        lhsT = zt_i.ap[:, hm, ds(128 * m, 128)]
                        rhs = pcat[:, hm, ds(K * (j - 1), K)]
                        _mm(nc, pos[:, m, 0, :], lhsT, rhs, hm == 0, hm == 1)
                ost = ostage.tile(
                    [128, 2, 1, K], F32, tag="ost0s", bufs=2, name=f"osts_{t0}"
                )
                _copy(nc, ceng, ost, pos)
                deng.dma_start(out=y_r[:, :, ds(t0, 1), :], in_=ost)

            # Three product chains P_j = (W^3)^T P_{j-3} (classes j mod 3)
            # run concurrently on PE with copies spread over DVE/ACT/Pool,
            # so P-powers land every ~0.45us instead of the ~1.3us
            # serial-chain cadence.
            q3 = _Mat(mats.tile([128, 2, K], F32R, tag="q3", name="q3"))
            chain_eng = {1: "v", 2: "s", 0: "v"}
            _product(nc, psz, P(2), q1, p1, "v")  # Wt^2
            _product(nc, psz, P(3), q1, P(2), "s")  # Wt^3
            emit_single(zt0, 1, 0, "v", nc.sync)  # needs only P1/Z0
            # (singles alternate SP / Pool-SWDGE queues: a single HWDGE
            # queue only sustains one DMA issue per ~1.2us)
            _product(nc, psz, q3, P(2), q1, "v")  # Q3 = W^2 W = W^3
            emit_single(zt0, 2, 1, "s", nc.sync)
            _product(nc, psz, P(4), q1, P(3), "v")  # Wt^4 sequentially --
            # lands ~1.3us before the Q3-chain could make it
            emit_single(zt0, 3, 2, "v", nc.sync)
            _product(nc, psz, P(5), q3, P(2), chain_eng[2])
            _product(nc, psz, P(6), q3, P(3), chain_eng[0])
            emit_single(zt0, 4, 3, "s", nc.sync)
            for j in range(7, 9):
                _product(nc, psz, P(j), q3, P(j - 3), chain_eng[j % 3])
            emit_pair(zt0, 5, 4)
            for j in range(9, 10):
                _product(nc, psz, P(j), q3, P(j - 3), chain_eng[j % 3])
            emit_pair(zt0, 7, 6)
            zt1 = _Mat(
                zts.tile([128, 2, K], F32R, tag="zt", bufs=zt_pool_bufs, name="zt1")
            )
            _product(nc, psz, zt1, P(8), zt0)  # Z1 = (Wt^8)^T Z0^T
            for j in range(10, 12):
                _product(nc, psz, P(j), q3, P(j - 3), chain_eng[j % 3])
            emit_pair(zt1, 1, 8)
            for j in range(12, 14):
                _product(nc, psz, P(j), q3, P(j - 3), chain_eng[j % 3])
            emit_pair(zt1, 3, 10)
            for j in range(14, 16):
                _product(nc, psz, P(j), q3, P(j - 3), chain_eng[j % 3])
            emit_pair(zt1, 5, 12)
            _product(nc, psz, P(S), q3, P(S - 3), chain_eng[S % 3])  # Wt^16
            a1 = P(S)
            zt2 = _Mat(
                zts.tile([128, 2, K], F32R, tag="zt", bufs=zt_pool_bufs, name="zt2")
            )
            _product(nc, psz, zt2, a1, zt1)  # Z2
            emit_pair(zt1, 7, 14)
            emit_pair(zt1, 9, 16)
            emit_pair(zt1, 11, 18)
            emit_outputs(zt1, 20, 13, 4)  # c1 tail: t=20..23 staged
            # --- steady state: Z_{i+1} = Z_i Wt^16, one 16-step staged
            # emission per checkpoint, emission order = checkpoint order.
            zt_prev = zt2
            for i in range(2, M + 1):
                t0 = 8 + S * (i - 1)
                w = S if i < M else T - t0  # final checkpoint covers 8
                emit_outputs(zt_prev, t0, 1, w)
                if i < M:
                    zt_next = _Mat(
                        zts.tile(
                            [128, 2, K], F32R, tag="zt", bufs=zt_pool_bufs, name=f"zt{i + 1}"
                        )
                    )
                    _product(nc, psz, zt_next, a1, zt_prev)
                    zt_prev = zt_next

    nc.compile()
    return nc


_cached_nc = None
_last_results = None


def kernel(x, W, T=None):
    global _cached_nc, _last_results
    if _cached_nc is None:
        _cached_nc = _build_program()
    nc = _cached_nc

    x2 = np.ascontiguousarray(np.asarray(x, dtype=np.float32).reshape(B_FULL, K))
    w2 = np.ascontiguousarray(np.asarray(W, dtype=np.float32))
    in_maps = [
        {"x": x2[i * B_SH : (i + 1) * B_SH], "w": w2} for i in range(N_CORES)
    ]
    res = run_bass_kernel_spmd(
        nc,
        in_maps,
        core_ids=list(range(N_CORES)),
        trace=bool(os.environ.get("BASS_TRACE")),
    )
    _last_results = res
    y = np.concatenate([res.results[i]["y"] for i in range(N_CORES)], axis=0)
    return y


# revision 78
# speedup vs baseline: 1.0341x; 1.0001x over previous
"""Trainium2 Bass kernel for the Koopman-operator rollout.

Reference computation: y0 = x[:, 0, :]  (shape [2048, 256]);
    y_t = y_{t-1} @ W.T  for t = 1..512, Y[:, t-1, :] = y_t.
Output: [2048, 512, 256] fp32 (1 GiB) -> memory-bound target; the whole
kernel is paced by the ~374 us it takes to write 128 MiB/core to HBM at
360 GB/s, so the design goal is a DMA stream with no idle slots.

Strategy (8 cores, data-parallel over batch, 256 rows/core):
  Let Wt = W.T.  Y[:, t] = y0 @ Wt^{t+1}.
  * P_j = Wt^j for j=1..16 held in one SBUF strip (pcat).  P2/P3 come
    off W^T directly; P4 off a short sequential hop; P5..P16 via THREE
    concurrent chains P_j = (W^3)^T P_{j-3} (helper Q3), so fresh
    powers land every ~0.45 us during the prologue instead of the
    ~1.3 us serial product cadence.
  * Offset-8 checkpointing: checkpoint 0 covers only t=0..7
    (Z1 = Z0 Wt^8 needs just P8); checkpoint i>=1 covers
    t = 8+16(i-1)..+15.  The P-chain construction then amortizes over
    24 prologue-emitted timesteps, keeping PE matmul work per DMA slot
    under the DMA cadence, and there is no staged-checkpoint seam at
    the end of c0.  Z_{i+1}^T = (Wt^16)^T Z_i^T advances sequentially
    (error growth checked against a 10-bit-mantissa numpy emulation:
    same as the prefix-doubling ladder, which emission-operand
    rounding dominates).
  * Prologue emissions are latency-shaped: four 1-step units (j=1 as
    soon as only P1/Z0 exist), then 2-step units gated pair-by-pair on
    the P-chains, staged via one PSUM bank + DVE/ACT drain copies and
    SP-issued DMAs.  Steady state: one 16-step staged tile + one 2 MiB
    DMA per batch half per checkpoint (supply runs ~2x ahead).
  * Warm-up: dummy PE matmuls from t~0 hold the tensor engine at full
    p-state before real work lands; a 1-element ACT copy hoists the
    1.3 us activation-table load off the critical path; W loads before
    x (the product chain hanging off W is longer).
  Matmul-operand tiles are float32r (full PE rate at N>=256, fp32 PSUM
  accumulation); PSUM->SBUF copies perform the f32->f32r rounding.

  Cost-model timeline: ~386 us/core vs the ~374 us HBM-write floor
  (startup DMA lead-in ~2 us + prologue fill ~8 us + tail sem ~1.6 us).
"""

import os

import numpy as np

import concourse.bass as bass
import concourse.mybir as mybir
import concourse.tile as tile
from concourse import bacc
from concourse.bass import ds
from concourse.bass_utils import run_bass_kernel_spmd
from concourse.masks import make_identity

F32 = mybir.dt.float32
F32R = mybir.dt.float32r

N_CORES = 8
B_FULL = 2048
B_SH = B_FULL // N_CORES  # 256 batch rows per core
K = 256  # state dim
T = 512  # time steps
S = 16  # timesteps per checkpoint chunk
M = T // S  # 32 checkpoints

# engine choice for PSUM->SBUF output copies: every Nth tile on ScalarE
ACT_COPY_EVERY = int(os.environ.get("K_ACT_EVERY", "2"))
N_DUMMY = int(os.environ.get("K_DUMMY", "6"))


def _mm(nc, out, lhsT, rhs, start, stop):
    # operands are float32r tiles already (producers round to f32r)
    nc.tensor.matmul(out, lhsT, rhs, start=start, stop=stop)


class _Mat:
    """A 256x256 matrix stored as an SBUF tile [128, 2, 256]:
    elem (p, h, c) = M[h*128 + p, c]."""

    def __init__(self, ap):
        self.ap = ap

    def half(self, hm):
        # [128, 256] slice: rows hm*128 .. hm*128+127 (partition = row)
        return self.ap[:, hm, :]

    def blk(self, hm, hc):
        # [128, 128] block: rows hm*128.., cols hc*128..
        return self.ap[:, hm, ds(128 * hc, 128)]

    def whole(self):
        # [128, 2, 256]: both row-halves (copy destination)
        return self.ap


_prod_ctr = [0]


def _product(nc, psum_pool, dst, lhsT_mat, rhs_mat, copy_eng=None):
    """dst = lhsT_mat.T @ rhs_mat  (all 256x256 _Mats).

    Both column-halves share one PSUM bank ([128, 2, 256]) and drain with a
    single copy (engine selectable), so a product costs 1 PSUM slot,
    4 matmuls, 1 copy."""
    _prod_ctr[0] += 1
    ps = psum_pool.tile([128, 2, 256], F32, tag="psz", name=f"psz_{_prod_ctr[0]}")
    for ha in range(2):
        for hm in range(2):
            _mm(nc, ps[:, ha, :], lhsT_mat.blk(hm, ha), rhs_mat.half(hm), hm == 0, hm == 1)
    _copy(nc, copy_eng or "v", dst.whole(), ps)


def _copy(nc, eng, dst, src):
    # NOTE: "g" (GPSIMD/Pool) cannot read PSUM on real HW -- only use it
    # for SBUF->SBUF copies.
    if eng == "v":
        nc.vector.tensor_copy(dst, src)
    elif eng == "g":
        nc.gpsimd.tensor_copy(dst, src)
    else:
        nc.scalar.copy(dst, src)


def _build_program():
    nc = bacc.Bacc(
        "TRN2",
        target_bir_lowering=False,
        debug=False,
        enable_asserts=False,
        num_devices=N_CORES,
    )
    x_d = nc.dram_tensor("x", [B_SH, K], F32, kind="ExternalInput").ap()
    w_d = nc.dram_tensor("w", [K, K], F32, kind="ExternalInput").ap()
    y_d = nc.dram_tensor("y", [B_SH, T, K], F32, kind="ExternalOutput").ap()

    with tile.TileContext(nc) as tc:
        with (
            tc.tile_pool(name="consts", bufs=1) as consts,
            tc.tile_pool(name="mats", bufs=1) as mats,
            tc.tile_pool(name="zts", bufs=1) as zts,
            tc.tile_pool(name="ostage", bufs=int(os.environ.get("K_OST", "3"))) as ostage,
            tc.tile_pool(name="pso", bufs=int(os.environ.get("K_PSO", "4")), space="PSUM") as pso,
            tc.tile_pool(name="psz", bufs=int(os.environ.get("K_PSZ", "4")), space="PSUM") as psz,
        ):
            ident = consts.tile([128, 128], F32, tag="ident", name="ident")
            make_identity(nc, ident)

            # --- PE clock warm-up: back-to-back dummy matmuls from t~0 so
            # the tensor engine is at full p-state when real work lands.
            for d in range(N_DUMMY):
                warm = psz.tile([128, 128], F32, tag="psz", name=f"warm_{d}")
                _mm(nc, warm, ident, ident, True, True)

            # --- prime the Activation engine's function table at t~0 so the
            # 1.3us LoadActFuncSet doesn't land in front of the first real
            # ACT copy on the critical path.
            actwarm = consts.tile([128, 1], F32, tag="actwarm", name="actwarm")
            nc.scalar.copy(actwarm, ident[:, 0:1])

            # --- inputs: W first on SP (the product chain hanging off W is
            # longer than x's transpose chain), x in parallel on ACT
            w_nat = consts.tile([128, 2, K], F32, tag="w_nat", name="w_nat")
            x_nat = consts.tile([128, 2, K], F32, tag="x_nat", name="x_nat")
            w_v = w_d.rearrange("(h p) k -> p h k", p=128)
            x_v = x_d.rearrange("(h p) k -> p h k", p=128)
            nc.sync.dma_start(out=w_nat, in_=w_v)
            nc.scalar.dma_start(out=x_nat, in_=x_v)

            # Pcat holds P_1..P_16 row-half-major: [128, 2, 16*256]
            pcat = mats.tile([128, 2, S * K], F32R, tag="pcat", name="pcat")

            def P(j):  # 1-indexed power as a _Mat-like view
                class V:
                    def half(self, hm, _j=j):
                        return pcat[:, hm, ds(K * (_j - 1), K)]

                    def blk(self, hm, hc, _j=j):
                        return pcat[:, hm, ds(K * (_j - 1) + 128 * hc, 128)]

                    def whole(self, _j=j):
                        return pcat[:, :, ds(K * (_j - 1), K)]

                return V()

            # --- transposes: P_1 = W^T first (W lands first), then
            # Z0^T = x^T; psum->sbuf copies split across DVE/ACT
            zt_pool_bufs = 3
            zt0 = _Mat(zts.tile([128, 2, K], F32R, tag="zt", bufs=zt_pool_bufs, name="zt0"))
            p1 = P(1)
            # both g-blocks of a half share one PSUM tile -> single drain copy
            for h in range(2):
                pst2 = psz.tile([128, 2, 128], F32, tag="psz", name=f"pstw_{h}")
                for g in range(2):
                    nc.tensor.transpose(
                        pst2[:, g, :], w_nat[:, g, ds(128 * h, 128)], ident
                    )
                _copy(nc, "v" if h == 0 else "s", pcat[:, h, ds(0, K)], pst2)
            for h in range(2):
                pst = psz.tile([128, 2, 128], F32, tag="psz", name=f"pstx_{h}")
                for g in range(2):
                    nc.tensor.transpose(
                        pst[:, g, :], x_nat[:, g, ds(128 * h, 128)], ident
                    )
                _copy(nc, "v" if h == 0 else "s", zt0.ap[:, h, :], pst)
            # q1 operand (rounded W, natural layout) on the otherwise-idle
            # Pool engine so it gates the first product as early as possible
            w_r = consts.tile([128, 2, K], F32R, tag="w_r", name="w_r")
            _copy(nc, "g", w_r, w_nat)
            q1 = _Mat(w_r)

            copy_ctr = [0]

            def pos_copy(dst, src, force_vector=False):
                if not force_vector and (
                    copy_ctr[0] % ACT_COPY_EVERY == ACT_COPY_EVERY - 1
                ):
                    nc.scalar.copy(dst, src)
                else:
                    nc.vector.tensor_copy(dst, src)
                copy_ctr[0] += 1

            y_r = y_d.rearrange("(h p) t k -> p h t k", p=128)

            pair_ctr = [0]

            def emit_pair(zt_i, jg, t0):
                """Y[:, t0 : t0+2] = Z_i @ [P_{jg} | P_{jg+1}] (jg is
                1-indexed).  Per-batch-half PSUM tiles, staged into one
                [128, 2, 2, K] tile (copies split DVE/ACT), one merged DMA
                on SP.  Used for the prologue stream (c0 + c1 front) where
                per-pair gating on the P-chain matters."""
                pair_ctr[0] += 1
                ost = ostage.tile(
                    [128, 2, 2, K], F32, tag="ost0", bufs=4, name=f"ostp_{t0}"
                )
                for m in range(2):
                    pos = pso.tile([128, 2, K], F32, tag="pso", name=f"posp_{m}_{t0}")
                    for hm in range(2):
                        lhsT = zt_i.ap[:, hm, ds(128 * m, 128)]
                        rhs = pcat[:, hm, ds(K * (jg - 1), 2 * K)]
                        _mm(nc, pos, lhsT, rhs, hm == 0, hm == 1)
                    _copy(nc, "v" if m == 0 else "s", ost[:, m, :, :], pos)
                nc.sync.dma_start(out=y_r[:, :, ds(t0, 2), :], in_=ost)

            def emit_outputs(zt_i, t0, j0, w):
                """Y[:, t0 : t0+w] = Z_i @ [P_{j0} .. P_{j0+w-1}], staged as
                one w-step SBUF tile + one DMA per batch half (steady
                state; supply runs far ahead of the DMA stream here)."""
                for m in range(2):  # batch half
                    ost = ostage.tile(
                        [128, w, K], F32, tag=f"ost{w}", name=f"ost_{t0}_{m}"
                    )
                    pos = {}
                    for n in range(w // 2):
                        pos[n] = pso.tile(
                            [128, 2, K], F32, tag="pso", name=f"pso_{t0}_{m}_{n}"
                        )
                    for hm in range(2):
                        lhsT = zt_i.ap[:, hm, ds(128 * m, 128)]
                        for n in pos:
                            rhs = pcat[:, hm, ds(K * (j0 - 1) + 512 * n, 512)]
                            _mm(nc, pos[n], lhsT, rhs, hm == 0, hm == 1)
                    for n in pos:
                        pos_copy(ost[:, ds(2 * n, 2), :], pos[n])
                    nc.sync.dma_start(
                        out=y_d[ds(128 * m, 128), ds(t0, w), :],
                        in_=ost,
                    )

            # --- Offset-8 checkpointing: checkpoint 0 covers only t=0..7
            # (Z1 = Z0 Wt^8 needs just P8), checkpoint i>=1 covers
            # t = 8+16(i-1) .. +15.  The P-chain construction (16 products)
            # then amortizes over 24 pair-granular emitted timesteps, which
            # brings PE work per 2-step DMA slot under the DMA cadence --
            # the stream never starves once it starts, and there is no
            # staged-checkpoint seam at the end of c0.
            # evens chain: P_{j+2} = Wt^2 P_j (lhsT = Q2); odds likewise.
            def emit_single(zt_i, j, t0, ceng, deng):
                """Y[:, t0] = Z_i @ P_j: both batch halves in one PSUM bank,
                one staging copy, one small DMA.  Prologue front only."""
                pos = pso.tile([128, 2, 1, K], F32, tag="pso", name=f"poss_{t0}")
                for m in range(2):
                    for hm in range(2):
                        lhsT = zt_i.ap[:, hm, ds(128 * m, 128)]
                        rhs = pcat[:, hm, ds(K * (j - 1), K)]
                        _mm(nc, pos[:, m, 0, :], lhsT, rhs, hm == 0, hm == 1)
                ost = ostage.tile(
                    [128, 2, 1, K], F32, tag="ost0s", bufs=2, name=f"osts_{t0}"
                )
                _copy(nc, ceng, ost, pos)
                deng.dma_start(out=y_r[:, :, ds(t0, 1), :], in_=ost)

            # Three product chains P_j = (W^3)^T P_{j-3} (classes j mod 3)
            # run concurrently on PE with copies spread over DVE/ACT/Pool,
            # so P-powers land every ~0.45us instead of the ~1.3us
            # serial-chain cadence.
            q3 = _Mat(mats.tile([128, 2, K], F32R, tag="q3", name="q3"))
            chain_eng = {1: "v", 2: "s", 0: "s"}
            _product(nc, psz, P(2), q1, p1, "v")  # Wt^2
            _product(nc, psz, P(3), q1, P(2), "s")  # Wt^3
            emit_single(zt0, 1, 0, "v", nc.sync)  # needs only P1/Z0
            # (singles alternate SP / Pool-SWDGE queues: a single HWDGE
            # queue only sustains one DMA issue per ~1.2us)
            _product(nc, psz, q3, P(2), q1, "v")  # Q3 = W^2 W = W^3
            emit_single(zt0, 2, 1, "s", nc.sync)
            _product(nc, psz, P(4), q1, P(3), "s")  # Wt^4 sequentially --
            # lands ~1.3us before the Q3-chain could make it
            emit_single(zt0, 3, 2, "v", nc.sync)
            _product(nc, psz, P(5), q3, P(2), chain_eng[2])
            _product(nc, psz, P(6), q3, P(3), chain_eng[0])
            emit_single(zt0, 4, 3, "s", nc.sync)
            for j in range(7, 9):
                _product(nc, psz, P(j), q3, P(j - 3), chain_eng[j % 3])
            emit_pair(zt0, 5, 4)
            for j in range(9, 10):
                _product(nc, psz, P(j), q3, P(j - 3), chain_eng[j % 3])
            emit_pair(zt0, 7, 6)
            zt1 = _Mat(
                zts.tile([128, 2, K], F32R, tag="zt", bufs=zt_pool_bufs, name="zt1")
            )
            _product(nc, psz, zt1, P(8), zt0)  # Z1 = (Wt^8)^T Z0^T
            for j in range(10, 12):
                _product(nc, psz, P(j), q3, P(j - 3), chain_eng[j % 3])
            emit_pair(zt1, 1, 8)
            for j in range(12, 14):
                _product(nc, psz, P(j), q3, P(j - 3), chain_eng[j % 3])
            emit_pair(zt1, 3, 10)
            for j in range(14, 16):
                _product(nc, psz, P(j), q3, P(j - 3), chain_eng[j % 3])
            emit_pair(zt1, 5, 12)
            _product(nc, psz, P(S), q3, P(S - 3), chain_eng[S % 3])  # Wt^16
            a1 = P(S)
            zt2 = _Mat(
                zts.tile([128, 2, K], F32R, tag="zt", bufs=zt_pool_bufs, name="zt2")
            )
            _product(nc, psz, zt2, a1, zt1)  # Z2
            emit_pair(zt1, 7, 14)
            emit_pair(zt1, 9, 16)
            emit_pair(zt1, 11, 18)
            emit_outputs(zt1, 20, 13, 4)  # c1 tail: t=20..23 staged
            # --- steady state: Z_{i+1} = Z_i Wt^16, one 16-step staged
            # emission per checkpoint, emission order = checkpoint order.
            zt_prev = zt2
            for i in range(2, M + 1):
                t0 = 8 + S * (i - 1)
                w = S if i < M else T - t0  # final checkpoint covers 8
                emit_outputs(zt_prev, t0, 1, w)
                if i < M:
                    zt_next = _Mat(
                        zts.tile(
                            [128, 2, K], F32R, tag="zt", bufs=zt_pool_bufs, name=f"zt{i + 1}"
                        )
                    )
                    _product(nc, psz, zt_next, a1, zt_prev)
                    zt_prev = zt_next

    nc.compile()
    return nc


_cached_nc = None
_last_results = None


def kernel(x, W, T=None):
    global _cached_nc, _last_results
    if _cached_nc is None:
        _cached_nc = _build_program()
    nc = _cached_nc

    x2 = np.ascontiguousarray(np.asarray(x, dtype=np.float32).reshape(B_FULL, K))
    w2 = np.ascontiguousarray(np.asarray(W, dtype=np.float32))
    in_maps = [
        {"x": x2[i * B_SH : (i + 1) * B_SH], "w": w2} for i in range(N_CORES)
    ]
    res = run_bass_kernel_spmd(
        nc,
        in_maps,
        core_ids=list(range(N_CORES)),
        trace=bool(os.environ.get("BASS_TRACE")),
    )
    _last_results = res
    y = np.concatenate([res.results[i]["y"] for i in range(N_CORES)], axis=0)
    return y


# revision 81
# speedup vs baseline: 1.0343x; 1.0002x over previous
"""Trainium2 Bass kernel for the Koopman-operator rollout.

Reference computation: y0 = x[:, 0, :]  (shape [2048, 256]);
    y_t = y_{t-1} @ W.T  for t = 1..512, Y[:, t-1, :] = y_t.
Output: [2048, 512, 256] fp32 (1 GiB) -> memory-bound target; the whole
kernel is paced by the ~374 us it takes to write 128 MiB/core to HBM at
360 GB/s, so the design goal is a DMA stream with no idle slots.

Strategy (8 cores, data-parallel over batch, 256 rows/core):
  Let Wt = W.T.  Y[:, t] = y0 @ Wt^{t+1}.
  * P_j = Wt^j for j=1..16 held in one SBUF strip (pcat).  P2/P3 come
    off W^T directly; P4 off a short sequential hop; P5..P16 via THREE
    concurrent chains P_j = (W^3)^T P_{j-3} (helper Q3), so fresh
    powers land every ~0.45 us during the prologue instead of the
    ~1.3 us serial product cadence.
  * Offset-8 checkpointing: checkpoint 0 covers only t=0..7
    (Z1 = Z0 Wt^8 needs just P8); checkpoint i>=1 covers
    t = 8+16(i-1)..+15.  The P-chain construction then amortizes over
    24 prologue-emitted timesteps, keeping PE matmul work per DMA slot
    under the DMA cadence, and there is no staged-checkpoint seam at
    the end of c0.  Z_{i+1}^T = (Wt^16)^T Z_i^T advances sequentially
    (error growth checked against a 10-bit-mantissa numpy emulation:
    same as the prefix-doubling ladder, which emission-operand
    rounding dominates).
  * Prologue emissions are latency-shaped: four 1-step units (j=1 as
    soon as only P1/Z0 exist), then 2-step units gated pair-by-pair on
    the P-chains, staged via one PSUM bank + DVE/ACT drain copies and
    SP-issued DMAs.  Steady state: one 16-step staged tile + one 2 MiB
    DMA per batch half per checkpoint (supply runs ~2x ahead).
  * Warm-up: dummy PE matmuls from t~0 hold the tensor engine at full
    p-state before real work lands; a 1-element ACT copy hoists the
    1.3 us activation-table load off the critical path; W loads before
    x (the product chain hanging off W is longer).
  Matmul-operand tiles are float32r (full PE rate at N>=256, fp32 PSUM
  accumulation); PSUM->SBUF copies perform the f32->f32r rounding.

  Cost-model timeline: ~386 us/core vs the ~374 us HBM-write floor
  (startup DMA lead-in ~2 us + prologue fill ~8 us + tail sem ~1.6 us).
"""

import os

import numpy as np

import concourse.bass as bass
import concourse.mybir as mybir
import concourse.tile as tile
from concourse import bacc
from concourse.bass import ds
from concourse.bass_utils import run_bass_kernel_spmd
from concourse.masks import make_identity

F32 = mybir.dt.float32
F32R = mybir.dt.float32r

N_CORES = 8
B_FULL = 2048
B_SH = B_FULL // N_CORES  # 256 batch rows per core
K = 256  # state dim
T = 512  # time steps
S = 16  # timesteps per checkpoint chunk
M = T // S  # 32 checkpoints

# engine choice for PSUM->SBUF output copies: every Nth tile on ScalarE
ACT_COPY_EVERY = int(os.environ.get("K_ACT_EVERY", "2"))
N_DUMMY = int(os.environ.get("K_DUMMY", "5"))


def _mm(nc, out, lhsT, rhs, start, stop):
    # operands are float32r tiles already (producers round to f32r)
    nc.tensor.matmul(out, lhsT, rhs, start=start, stop=stop)


class _Mat:
    """A 256x256 matrix stored as an SBUF tile [128, 2, 256]:
    elem (p, h, c) = M[h*128 + p, c]."""

    def __init__(self, ap):
        self.ap = ap

    def half(self, hm):
        # [128, 256] slice: rows hm*128 .. hm*128+127 (partition = row)
        return self.ap[:, hm, :]

    def blk(self, hm, hc):
        # [128, 128] block: rows hm*128.., cols hc*128..
        return self.ap[:, hm, ds(128 * hc, 128)]

    def whole(self):
        # [128, 2, 256]: both row-halves (copy destination)
        return self.ap


_prod_ctr = [0]


def _product(nc, psum_pool, dst, lhsT_mat, rhs_mat, copy_eng=None):
    """dst = lhsT_mat.T @ rhs_mat  (all 256x256 _Mats).

    Both column-halves share one PSUM bank ([128, 2, 256]) and drain with a
    single copy (engine selectable), so a product costs 1 PSUM slot,
    4 matmuls, 1 copy."""
    _prod_ctr[0] += 1
    ps = psum_pool.tile([128, 2, 256], F32, tag="psz", name=f"psz_{_prod_ctr[0]}")
    for ha in range(2):
        for hm in range(2):
            _mm(nc, ps[:, ha, :], lhsT_mat.blk(hm, ha), rhs_mat.half(hm), hm == 0, hm == 1)
    _copy(nc, copy_eng or "v", dst.whole(), ps)


def _copy(nc, eng, dst, src):
    # NOTE: "g" (GPSIMD/Pool) cannot read PSUM on real HW -- only use it
    # for SBUF->SBUF copies.
    if eng == "v":
        nc.vector.tensor_copy(dst, src)
    elif eng == "g":
        nc.gpsimd.tensor_copy(dst, src)
    else:
        nc.scalar.copy(dst, src)


def _build_program():
    nc = bacc.Bacc(
        "TRN2",
        target_bir_lowering=False,
        debug=False,
        enable_asserts=False,
        num_devices=N_CORES,
    )
    x_d = nc.dram_tensor("x", [B_SH, K], F32, kind="ExternalInput").ap()
    w_d = nc.dram_tensor("w", [K, K], F32, kind="ExternalInput").ap()
    y_d = nc.dram_tensor("y", [B_SH, T, K], F32, kind="ExternalOutput").ap()

    with tile.TileContext(nc) as tc:
        with (
            tc.tile_pool(name="consts", bufs=1) as consts,
            tc.tile_pool(name="mats", bufs=1) as mats,
            tc.tile_pool(name="zts", bufs=1) as zts,
            tc.tile_pool(name="ostage", bufs=int(os.environ.get("K_OST", "3"))) as ostage,
            tc.tile_pool(name="pso", bufs=int(os.environ.get("K_PSO", "4")), space="PSUM") as pso,
            tc.tile_pool(name="psz", bufs=int(os.environ.get("K_PSZ", "4")), space="PSUM") as psz,
        ):
            ident = consts.tile([128, 128], F32, tag="ident", name="ident")
            make_identity(nc, ident)

            # --- PE clock warm-up: back-to-back dummy matmuls from t~0 so
            # the tensor engine is at full p-state when real work lands.
            for d in range(N_DUMMY):
                warm = psz.tile([128, 128], F32, tag="psz", name=f"warm_{d}")
                _mm(nc, warm, ident, ident, True, True)

            # --- prime the Activation engine's function table at t~0 so the
            # 1.3us LoadActFuncSet doesn't land in front of the first real
            # ACT copy on the critical path.
            actwarm = consts.tile([128, 1], F32, tag="actwarm", name="actwarm")
            nc.scalar.copy(actwarm, ident[:, 0:1])

            # --- inputs: W first on SP (the product chain hanging off W is
            # longer than x's transpose chain), x in parallel on ACT
            w_nat = consts.tile([128, 2, K], F32, tag="w_nat", name="w_nat")
            x_nat = consts.tile([128, 2, K], F32, tag="x_nat", name="x_nat")
            w_v = w_d.rearrange("(h p) k -> p h k", p=128)
            x_v = x_d.rearrange("(h p) k -> p h k", p=128)
            nc.sync.dma_start(out=w_nat, in_=w_v)
            nc.scalar.dma_start(out=x_nat, in_=x_v)

            # Pcat holds P_1..P_16 row-half-major: [128, 2, 16*256]
            pcat = mats.tile([128, 2, S * K], F32R, tag="pcat", name="pcat")

            def P(j):  # 1-indexed power as a _Mat-like view
                class V:
                    def half(self, hm, _j=j):
                        return pcat[:, hm, ds(K * (_j - 1), K)]

                    def blk(self, hm, hc, _j=j):
                        return pcat[:, hm, ds(K * (_j - 1) + 128 * hc, 128)]

                    def whole(self, _j=j):
                        return pcat[:, :, ds(K * (_j - 1), K)]

                return V()

            # --- transposes: P_1 = W^T first (W lands first), then
            # Z0^T = x^T; psum->sbuf copies split across DVE/ACT
            zt_pool_bufs = 3
            zt0 = _Mat(zts.tile([128, 2, K], F32R, tag="zt", bufs=zt_pool_bufs, name="zt0"))
            p1 = P(1)
            # both g-blocks of a half share one PSUM tile -> single drain copy
            for h in range(2):
                pst2 = psz.tile([128, 2, 128], F32, tag="psz", name=f"pstw_{h}")
                for g in range(2):
                    nc.tensor.transpose(
                        pst2[:, g, :], w_nat[:, g, ds(128 * h, 128)], ident
                    )
                _copy(nc, "v" if h == 0 else "s", pcat[:, h, ds(0, K)], pst2)
            for h in range(2):
                pst = psz.tile([128, 2, 128], F32, tag="psz", name=f"pstx_{h}")
                for g in range(2):
                    nc.tensor.transpose(
                        pst[:, g, :], x_nat[:, g, ds(128 * h, 128)], ident
                    )
                _copy(nc, "v" if h == 0 else "s", zt0.ap[:, h, :], pst)
            # q1 operand (rounded W, natural layout) on the otherwise-idle
            # Pool engine so it gates the first product as early as possible
            w_r = consts.tile([128, 2, K], F32R, tag="w_r", name="w_r")
            _copy(nc, "g", w_r, w_nat)
            q1 = _Mat(w_r)

            copy_ctr = [0]

            def pos_copy(dst, src, force_vector=False):
                if not force_vector and (
                    copy_ctr[0] % ACT_COPY_EVERY == ACT_COPY_EVERY - 1
                ):
                    nc.scalar.copy(dst, src)
                else:
                    nc.vector.tensor_copy(dst, src)
                copy_ctr[0] += 1

            y_r = y_d.rearrange("(h p) t k -> p h t k", p=128)

            pair_ctr = [0]

            def emit_pair(zt_i, jg, t0):
                """Y[:, t0 : t0+2] = Z_i @ [P_{jg} | P_{jg+1}] (jg is
                1-indexed).  Per-batch-half PSUM tiles, staged into one
                [128, 2, 2, K] tile (copies split DVE/ACT), one merged DMA
                on SP.  Used for the prologue stream (c0 + c1 front) where
                per-pair gating on the P-chain matters."""
                pair_ctr[0] += 1
                ost = ostage.tile(
                    [128, 2, 2, K], F32, tag="ost0", bufs=4, name=f"ostp_{t0}"
                )
                for m in range(2):
                    pos = pso.tile([128, 2, K], F32, tag="pso", name=f"posp_{m}_{t0}")
                    for hm in range(2):
                        lhsT = zt_i.ap[:, hm, ds(128 * m, 128)]
                        rhs = pcat[:, hm, ds(K * (jg - 1), 2 * K)]
                        _mm(nc, pos, lhsT, rhs, hm == 0, hm == 1)
                    _copy(nc, "v" if m == 0 else "s", ost[:, m, :, :], pos)
                nc.sync.dma_start(out=y_r[:, :, ds(t0, 2), :], in_=ost)

            def emit_outputs(zt_i, t0, j0, w):
                """Y[:, t0 : t0+w] = Z_i @ [P_{j0} .. P_{j0+w-1}], staged as
                one w-step SBUF tile + one DMA per batch half (steady
                state; supply runs far ahead of the DMA stream here)."""
                for m in range(2):  # batch half
                    ost = ostage.tile(
                        [128, w, K], F32, tag=f"ost{w}", name=f"ost_{t0}_{m}"
                    )
                    pos = {}
                    for n in range(w // 2):
                        pos[n] = pso.tile(
                            [128, 2, K], F32, tag="pso", name=f"pso_{t0}_{m}_{n}"
                        )
                    for hm in range(2):
                        lhsT = zt_i.ap[:, hm, ds(128 * m, 128)]
                        for n in pos:
                            rhs = pcat[:, hm, ds(K * (j0 - 1) + 512 * n, 512)]
                            _mm(nc, pos[n], lhsT, rhs, hm == 0, hm == 1)
                    for n in pos:
                        pos_copy(ost[:, ds(2 * n, 2), :], pos[n])
                    nc.sync.dma_start(
                        out=y_d[ds(128 * m, 128), ds(t0, w), :],
                        in_=ost,
                    )

            # --- Offset-8 checkpointing: checkpoint 0 covers only t=0..7
            # (Z1 = Z0 Wt^8 needs just P8), checkpoint i>=1 covers
            # t = 8+16(i-1) .. +15.  The P-chain construction (16 products)
            # then amortizes over 24 pair-granular emitted timesteps, which
            # brings PE work per 2-step DMA slot under the DMA cadence --
            # the stream never starves once it starts, and there is no
            # staged-checkpoint seam at the end of c0.
            # evens chain: P_{j+2} = Wt^2 P_j (lhsT = Q2); odds likewise.
            def emit_single(zt_i, j, t0, ceng, deng):
                """Y[:, t0] = Z_i @ P_j: both batch halves in one PSUM bank,
                one staging copy, one small DMA.  Prologue front only."""
                pos = pso.tile([128, 2, 1, K], F32, tag="pso", name=f"poss_{t0}")
                for m in range(2):
                    for hm in range(2):
                        lhsT = zt_i.ap[:, hm, ds(128 * m, 128)]
                        rhs = pcat[:, hm, ds(K * (j - 1), K)]
                        _mm(nc, pos[:, m, 0, :], lhsT, rhs, hm == 0, hm == 1)
                ost = ostage.tile(
                    [128, 2, 1, K], F32, tag="ost0s", bufs=2, name=f"osts_{t0}"
                )
                _copy(nc, ceng, ost, pos)
                deng.dma_start(out=y_r[:, :, ds(t0, 1), :], in_=ost)

            # Three product chains P_j = (W^3)^T P_{j-3} (classes j mod 3)
            # run concurrently on PE with copies spread over DVE/ACT/Pool,
            # so P-powers land every ~0.45us instead of the ~1.3us
            # serial-chain cadence.
            q3 = _Mat(mats.tile([128, 2, K], F32R, tag="q3", name="q3"))
            chain_eng = {1: "v", 2: "s", 0: "s"}
            _product(nc, psz, P(2), q1, p1, "v")  # Wt^2
            _product(nc, psz, P(3), q1, P(2), "s")  # Wt^3
            emit_single(zt0, 1, 0, "v", nc.sync)  # needs only P1/Z0
            # (singles alternate SP / Pool-SWDGE queues: a single HWDGE
            # queue only sustains one DMA issue per ~1.2us)
            _product(nc, psz, q3, P(2), q1, "v")  # Q3 = W^2 W = W^3
            emit_single(zt0, 2, 1, "s", nc.sync)
            _product(nc, psz, P(4), q1, P(3), "s")  # Wt^4 sequentially --
            # lands ~1.3us before the Q3-chain could make it
            emit_single(zt0, 3, 2, "v", nc.sync)
            _product(nc, psz, P(5), q3, P(2), chain_eng[2])
            _product(nc, psz, P(6), q3, P(3), chain_eng[0])
            emit_single(zt0, 4, 3, "s", nc.sync)
            for j in range(7, 9):
                _product(nc, psz, P(j), q3, P(j - 3), chain_eng[j % 3])
            emit_pair(zt0, 5, 4)
            for j in range(9, 10):
                _product(nc, psz, P(j), q3, P(j - 3), chain_eng[j % 3])
            emit_pair(zt0, 7, 6)
            zt1 = _Mat(
                zts.tile([128, 2, K], F32R, tag="zt", bufs=zt_pool_bufs, name="zt1")
            )
            _product(nc, psz, zt1, P(8), zt0)  # Z1 = (Wt^8)^T Z0^T
            for j in range(10, 12):
                _product(nc, psz, P(j), q3, P(j - 3), chain_eng[j % 3])
            emit_pair(zt1, 1, 8)
            for j in range(12, 14):
                _product(nc, psz, P(j), q3, P(j - 3), chain_eng[j % 3])
            emit_pair(zt1, 3, 10)
            for j in range(14, 16):
                _product(nc, psz, P(j), q3, P(j - 3), chain_eng[j % 3])
            emit_pair(zt1, 5, 12)
            _product(nc, psz, P(S), q3, P(S - 3), chain_eng[S % 3])  # Wt^16
            a1 = P(S)
            zt2 = _Mat(
                zts.tile([128, 2, K], F32R, tag="zt", bufs=zt_pool_bufs, name="zt2")
            )
            _product(nc, psz, zt2, a1, zt1)  # Z2
            emit_pair(zt1, 7, 14)
            emit_pair(zt1, 9, 16)
            emit_pair(zt1, 11, 18)
            emit_outputs(zt1, 20, 13, 4)  # c1 tail: t=20..23 staged
            # --- steady state: Z_{i+1} = Z_i Wt^16, one 16-step staged
            # emission per checkpoint, emission order = checkpoint order.
            zt_prev = zt2
            for i in range(2, M + 1):
                t0 = 8 + S * (i - 1)
                w = S if i < M else T - t0  # final checkpoint covers 8
                emit_outputs(zt_prev, t0, 1, w)
                if i < M:
                    zt_next = _Mat(
                        zts.tile(
                            [128, 2, K], F32R, tag="zt", bufs=zt_pool_bufs, name=f"zt{i + 1}"
                        )
                    )
                    _product(nc, psz, zt_next, a1, zt_prev)
                    zt_prev = zt_next

    nc.compile()
    return nc


_cached_nc = None
_last_results = None


def kernel(x, W, T=None):
    global _cached_nc, _last_results
    if _cached_nc is None:
        _cached_nc = _build_program()
    nc = _cached_nc

    x2 = np.ascontiguousarray(np.asarray(x, dtype=np.float32).reshape(B_FULL, K))
    w2 = np.ascontiguousarray(np.asarray(W, dtype=np.float32))
    in_maps = [
        {"x": x2[i * B_SH : (i + 1) * B_SH], "w": w2} for i in range(N_CORES)
    ]
    res = run_bass_kernel_spmd(
        nc,
        in_maps,
        core_ids=list(range(N_CORES)),
        trace=bool(os.environ.get("BASS_TRACE")),
    )
    _last_results = res
    y = np.concatenate([res.results[i]["y"] for i in range(N_CORES)], axis=0)
    return y


# revision 90
# speedup vs baseline: 1.0345x; 1.0003x over previous
"""Trainium2 Bass kernel for the Koopman-operator rollout.

Reference computation: y0 = x[:, 0, :]  (shape [2048, 256]);
    y_t = y_{t-1} @ W.T  for t = 1..512, Y[:, t-1, :] = y_t.
Output: [2048, 512, 256] fp32 (1 GiB) -> memory-bound target; the whole
kernel is paced by the ~374 us it takes to write 128 MiB/core to HBM at
360 GB/s, so the design goal is a DMA stream with no idle slots.

Strategy (8 cores, data-parallel over batch, 256 rows/core):
  Let Wt = W.T.  Y[:, t] = y0 @ Wt^{t+1}.
  * P_j = Wt^j for j=1..16 held in one SBUF strip (pcat).  P2/P3 come
    off W^T directly; P4 off a short sequential hop; P5..P16 via THREE
    concurrent chains P_j = (W^3)^T P_{j-3} (helper Q3), so fresh
    powers land every ~0.45 us during the prologue instead of the
    ~1.3 us serial product cadence.
  * Offset-8 checkpointing: checkpoint 0 covers only t=0..7
    (Z1 = Z0 Wt^8 needs just P8); checkpoint i>=1 covers
    t = 8+16(i-1)..+15.  The P-chain construction then amortizes over
    24 prologue-emitted timesteps, keeping PE matmul work per DMA slot
    under the DMA cadence, and there is no staged-checkpoint seam at
    the end of c0.  Z_{i+1}^T = (Wt^16)^T Z_i^T advances sequentially
    (error growth checked against a 10-bit-mantissa numpy emulation:
    same as the prefix-doubling ladder, which emission-operand
    rounding dominates).
  * Prologue emissions are latency-shaped: four 1-step units (j=1 as
    soon as only P1/Z0 exist), then 2-step units gated pair-by-pair on
    the P-chains, staged via one PSUM bank + DVE/ACT drain copies and
    SP-issued DMAs.  Steady state: one 16-step staged tile + one 2 MiB
    DMA per batch half per checkpoint (supply runs ~2x ahead).
  * Warm-up: dummy PE matmuls from t~0 hold the tensor engine at full
    p-state before real work lands; a 1-element ACT copy hoists the
    1.3 us activation-table load off the critical path; W loads before
    x (the product chain hanging off W is longer).
  Matmul-operand tiles are float32r (full PE rate at N>=256, fp32 PSUM
  accumulation); PSUM->SBUF copies perform the f32->f32r rounding.

  Cost-model timeline: ~386 us/core vs the ~374 us HBM-write floor
  (startup DMA lead-in ~2 us + prologue fill ~8 us + tail sem ~1.6 us).
"""

import os

import numpy as np

import concourse.bass as bass
import concourse.mybir as mybir
import concourse.tile as tile
from concourse import bacc
from concourse.bass import ds
from concourse.bass_utils import run_bass_kernel_spmd
from concourse.masks import make_identity

F32 = mybir.dt.float32
F32R = mybir.dt.float32r

N_CORES = 8
B_FULL = 2048
B_SH = B_FULL // N_CORES  # 256 batch rows per core
K = 256  # state dim
T = 512  # time steps
S = 16  # timesteps per checkpoint chunk
M = T // S  # 32 checkpoints

# engine choice for PSUM->SBUF output copies: every Nth tile on ScalarE
ACT_COPY_EVERY = int(os.environ.get("K_ACT_EVERY", "2"))
N_DUMMY = int(os.environ.get("K_DUMMY", "5"))


def _mm(nc, out, lhsT, rhs, start, stop):
    # operands are float32r tiles already (producers round to f32r)
    nc.tensor.matmul(out, lhsT, rhs, start=start, stop=stop)


class _Mat:
    """A 256x256 matrix stored as an SBUF tile [128, 2, 256]:
    elem (p, h, c) = M[h*128 + p, c]."""

    def __init__(self, ap):
        self.ap = ap

    def half(self, hm):
        # [128, 256] slice: rows hm*128 .. hm*128+127 (partition = row)
        return self.ap[:, hm, :]

    def blk(self, hm, hc):
        # [128, 128] block: rows hm*128.., cols hc*128..
        return self.ap[:, hm, ds(128 * hc, 128)]

    def whole(self):
        # [128, 2, 256]: both row-halves (copy destination)
        return self.ap


_prod_ctr = [0]


def _product(nc, psum_pool, dst, lhsT_mat, rhs_mat, copy_eng=None):
    """dst = lhsT_mat.T @ rhs_mat  (all 256x256 _Mats).

    Both column-halves share one PSUM bank ([128, 2, 256]) and drain with a
    single copy (engine selectable), so a product costs 1 PSUM slot,
    4 matmuls, 1 copy."""
    _prod_ctr[0] += 1
    ps = psum_pool.tile([128, 2, 256], F32, tag="psz", name=f"psz_{_prod_ctr[0]}")
    for ha in range(2):
        for hm in range(2):
            _mm(nc, ps[:, ha, :], lhsT_mat.blk(hm, ha), rhs_mat.half(hm), hm == 0, hm == 1)
    _copy(nc, copy_eng or "v", dst.whole(), ps)


def _copy(nc, eng, dst, src):
    # NOTE: "g" (GPSIMD/Pool) cannot read PSUM on real HW -- only use it
    # for SBUF->SBUF copies.
    if eng == "v":
        nc.vector.tensor_copy(dst, src)
    elif eng == "g":
        nc.gpsimd.tensor_copy(dst, src)
    else:
        nc.scalar.copy(dst, src)


def _build_program():
    nc = bacc.Bacc(
        "TRN2",
        target_bir_lowering=False,
        debug=False,
        enable_asserts=False,
        num_devices=N_CORES,
    )
    x_d = nc.dram_tensor("x", [B_SH, K], F32, kind="ExternalInput").ap()
    w_d = nc.dram_tensor("w", [K, K], F32, kind="ExternalInput").ap()
    y_d = nc.dram_tensor("y", [B_SH, T, K], F32, kind="ExternalOutput").ap()

    with tile.TileContext(nc) as tc:
        with (
            tc.tile_pool(name="consts", bufs=1) as consts,
            tc.tile_pool(name="mats", bufs=1) as mats,
            tc.tile_pool(name="zts", bufs=1) as zts,
            tc.tile_pool(name="ostage", bufs=int(os.environ.get("K_OST", "3"))) as ostage,
            tc.tile_pool(name="pso", bufs=int(os.environ.get("K_PSO", "4")), space="PSUM") as pso,
            tc.tile_pool(name="psz", bufs=int(os.environ.get("K_PSZ", "4")), space="PSUM") as psz,
        ):
            ident = consts.tile([128, 128], F32, tag="ident", name="ident")
            make_identity(nc, ident)

            # --- PE clock warm-up: back-to-back dummy matmuls from t~0 so
            # the tensor engine is at full p-state when real work lands.
            for d in range(N_DUMMY):
                warm = psz.tile([128, 128], F32, tag="psz", name=f"warm_{d}")
                _mm(nc, warm, ident, ident, True, True)

            # --- prime the Activation engine's function table at t~0 so the
            # 1.3us LoadActFuncSet doesn't land in front of the first real
            # ACT copy on the critical path.
            actwarm = consts.tile([128, 1], F32, tag="actwarm", name="actwarm")
            nc.scalar.copy(actwarm, ident[:, 0:1])

            # --- inputs: W first on SP (the product chain hanging off W is
            # longer than x's transpose chain), x in parallel on ACT
            w_nat = consts.tile([128, 2, K], F32, tag="w_nat", name="w_nat")
            x_nat = consts.tile([128, 2, K], F32, tag="x_nat", name="x_nat")
            w_v = w_d.rearrange("(h p) k -> p h k", p=128)
            x_v = x_d.rearrange("(h p) k -> p h k", p=128)
            nc.sync.dma_start(out=w_nat, in_=w_v)
            nc.scalar.dma_start(out=x_nat, in_=x_v)

            # Pcat holds P_1..P_16 row-half-major: [128, 2, 16*256]
            pcat = mats.tile([128, 2, S * K], F32R, tag="pcat", name="pcat")

            def P(j):  # 1-indexed power as a _Mat-like view
                class V:
                    def half(self, hm, _j=j):
                        return pcat[:, hm, ds(K * (_j - 1), K)]

                    def blk(self, hm, hc, _j=j):
                        return pcat[:, hm, ds(K * (_j - 1) + 128 * hc, 128)]

                    def whole(self, _j=j):
                        return pcat[:, :, ds(K * (_j - 1), K)]

                return V()

            # --- transposes: P_1 = W^T first (W lands first), then
            # Z0^T = x^T; psum->sbuf copies split across DVE/ACT
            zt_pool_bufs = 3
            zt0 = _Mat(zts.tile([128, 2, K], F32R, tag="zt", bufs=zt_pool_bufs, name="zt0"))
            p1 = P(1)
            # both g-blocks of a half share one PSUM tile -> single drain copy
            for h in range(2):
                pst2 = psz.tile([128, 2, 128], F32, tag="psz", name=f"pstw_{h}")
                for g in range(2):
                    nc.tensor.transpose(
                        pst2[:, g, :], w_nat[:, g, ds(128 * h, 128)], ident
                    )
                _copy(nc, "v" if h == 0 else "s", pcat[:, h, ds(0, K)], pst2)
            for h in range(2):
                pst = psz.tile([128, 2, 128], F32, tag="psz", name=f"pstx_{h}")
                for g in range(2):
                    nc.tensor.transpose(
                        pst[:, g, :], x_nat[:, g, ds(128 * h, 128)], ident
                    )
                _copy(nc, "v" if h == 0 else "s", zt0.ap[:, h, :], pst)
            # q1 operand (rounded W, natural layout) on the otherwise-idle
            # Pool engine so it gates the first product as early as possible
            w_r = consts.tile([128, 2, K], F32R, tag="w_r", name="w_r")
            _copy(nc, "g", w_r, w_nat)
            q1 = _Mat(w_r)

            copy_ctr = [0]

            def pos_copy(dst, src, force_vector=False):
                if not force_vector and (
                    copy_ctr[0] % ACT_COPY_EVERY == ACT_COPY_EVERY - 1
                ):
                    nc.scalar.copy(dst, src)
                else:
                    nc.vector.tensor_copy(dst, src)
                copy_ctr[0] += 1

            y_r = y_d.rearrange("(h p) t k -> p h t k", p=128)

            pair_ctr = [0]

            def emit_pair(zt_i, jg, t0):
                """Y[:, t0 : t0+2] = Z_i @ [P_{jg} | P_{jg+1}] (jg is
                1-indexed).  Per-batch-half PSUM tiles, staged into one
                [128, 2, 2, K] tile (copies split DVE/ACT), one merged DMA
                on SP.  Used for the prologue stream (c0 + c1 front) where
                per-pair gating on the P-chain matters."""
                pair_ctr[0] += 1
                ost = ostage.tile(
                    [128, 2, 2, K], F32, tag="ost0", bufs=4, name=f"ostp_{t0}"
                )
                for m in range(2):
                    pos = pso.tile([128, 2, K], F32, tag="pso", name=f"posp_{m}_{t0}")
                    for hm in range(2):
                        lhsT = zt_i.ap[:, hm, ds(128 * m, 128)]
                        rhs = pcat[:, hm, ds(K * (jg - 1), 2 * K)]
                        _mm(nc, pos, lhsT, rhs, hm == 0, hm == 1)
                    _copy(nc, "v" if m == 0 else "s", ost[:, m, :, :], pos)
                nc.sync.dma_start(out=y_r[:, :, ds(t0, 2), :], in_=ost)

            def emit_outputs(zt_i, t0, j0, w, groups=1):
                """Y[:, t0 : t0+w] = Z_i @ [P_{j0} .. P_{j0+w-1}], staged as
                one w-step SBUF tile + one DMA per batch half (steady
                state; supply runs far ahead of the DMA stream here).
                groups=2 interleaves matmuls/copies/DMA per half-group
                (lower latency entering steady state)."""
                npg = w // 2 // groups
                for m in range(2):  # batch half
                    ost = ostage.tile(
                        [128, w, K], F32, tag=f"ost{w}", name=f"ost_{t0}_{m}"
                    )
                    for g in range(groups):
                        pos = {}
                        for n in range(g * npg, (g + 1) * npg):
                            pos[n] = pso.tile(
                                [128, 2, K], F32, tag="pso", name=f"pso_{t0}_{m}_{n}"
                            )
                        for hm in range(2):
                            lhsT = zt_i.ap[:, hm, ds(128 * m, 128)]
                            for n in pos:
                                rhs = pcat[:, hm, ds(K * (j0 - 1) + 512 * n, 512)]
                                _mm(nc, pos[n], lhsT, rhs, hm == 0, hm == 1)
                        for n in pos:
                            pos_copy(ost[:, ds(2 * n, 2), :], pos[n])
                        nc.sync.dma_start(
                            out=y_d[
                                ds(128 * m, 128),
                                ds(t0 + 2 * npg * g, 2 * npg),
                                :,
                            ],
                            in_=ost[:, ds(2 * npg * g, 2 * npg), :],
                        )

            # --- Offset-8 checkpointing: checkpoint 0 covers only t=0..7
            # (Z1 = Z0 Wt^8 needs just P8), checkpoint i>=1 covers
            # t = 8+16(i-1) .. +15.  The P-chain construction (16 products)
            # then amortizes over 24 pair-granular emitted timesteps, which
            # brings PE work per 2-step DMA slot under the DMA cadence --
            # the stream never starves once it starts, and there is no
            # staged-checkpoint seam at the end of c0.
            # evens chain: P_{j+2} = Wt^2 P_j (lhsT = Q2); odds likewise.
            def emit_single(zt_i, j, t0, ceng, deng):
                """Y[:, t0] = Z_i @ P_j: both batch halves in one PSUM bank,
                one staging copy, one small DMA.  Prologue front only."""
                pos = pso.tile([128, 2, 1, K], F32, tag="pso", name=f"poss_{t0}")
                for m in range(2):
                    for hm in range(2):
                        lhsT = zt_i.ap[:, hm, ds(128 * m, 128)]
                        rhs = pcat[:, hm, ds(K * (j - 1), K)]
                        _mm(nc, pos[:, m, 0, :], lhsT, rhs, hm == 0, hm == 1)
                ost = ostage.tile(
                    [128, 2, 1, K], F32, tag="ost0s", bufs=2, name=f"osts_{t0}"
                )
                _copy(nc, ceng, ost, pos)
                deng.dma_start(out=y_r[:, :, ds(t0, 1), :], in_=ost)

            # Three product chains P_j = (W^3)^T P_{j-3} (classes j mod 3)
            # run concurrently on PE with copies spread over DVE/ACT/Pool,
            # so P-powers land every ~0.45us instead of the ~1.3us
            # serial-chain cadence.
            q3 = _Mat(mats.tile([128, 2, K], F32R, tag="q3", name="q3"))
            chain_eng = {1: "v", 2: "s", 0: "s"}
            _product(nc, psz, P(2), q1, p1, "v")  # Wt^2
            _product(nc, psz, P(3), q1, P(2), "s")  # Wt^3
            emit_single(zt0, 1, 0, "v", nc.sync)  # needs only P1/Z0
            # (singles alternate SP / Pool-SWDGE queues: a single HWDGE
            # queue only sustains one DMA issue per ~1.2us)
            _product(nc, psz, q3, P(2), q1, "v")  # Q3 = W^2 W = W^3
            emit_single(zt0, 2, 1, "s", nc.sync)
            _product(nc, psz, P(4), q1, P(3), "s")  # Wt^4 sequentially --
            # lands ~1.3us before the Q3-chain could make it
            emit_single(zt0, 3, 2, "v", nc.sync)
            _product(nc, psz, P(5), q3, P(2), chain_eng[2])
            _product(nc, psz, P(6), q3, P(3), chain_eng[0])
            emit_single(zt0, 4, 3, "s", nc.sync)
            for j in range(7, 9):
                _product(nc, psz, P(j), q3, P(j - 3), chain_eng[j % 3])
            emit_pair(zt0, 5, 4)
            for j in range(9, 10):
                _product(nc, psz, P(j), q3, P(j - 3), chain_eng[j % 3])
            emit_pair(zt0, 7, 6)
            zt1 = _Mat(
                zts.tile([128, 2, K], F32R, tag="zt", bufs=zt_pool_bufs, name="zt1")
            )
            _product(nc, psz, zt1, P(8), zt0)  # Z1 = (Wt^8)^T Z0^T
            for j in range(10, 12):
                _product(nc, psz, P(j), q3, P(j - 3), chain_eng[j % 3])
            emit_pair(zt1, 1, 8)
            for j in range(12, 14):
                _product(nc, psz, P(j), q3, P(j - 3), chain_eng[j % 3])
            emit_pair(zt1, 3, 10)
            for j in range(14, 16):
                _product(nc, psz, P(j), q3, P(j - 3), chain_eng[j % 3])
            emit_pair(zt1, 5, 12)
            _product(nc, psz, P(S), q3, P(S - 3), chain_eng[S % 3])  # Wt^16
            a1 = P(S)
            zt2 = _Mat(
                zts.tile([128, 2, K], F32R, tag="zt", bufs=zt_pool_bufs, name="zt2")
            )
            _product(nc, psz, zt2, a1, zt1)  # Z2
            emit_pair(zt1, 7, 14)
            emit_pair(zt1, 9, 16)
            emit_pair(zt1, 11, 18)
            emit_outputs(zt1, 20, 13, 4)  # c1 tail: t=20..23 staged
            # --- steady state: Z_{i+1} = Z_i Wt^16, one 16-step staged
            # emission per checkpoint, emission order = checkpoint order.
            zt_prev = zt2
            for i in range(2, M + 1):
                t0 = 8 + S * (i - 1)
                w = S if i < M else T - t0  # final checkpoint covers 8
                emit_outputs(zt_prev, t0, 1, w, groups=2 if i == 2 else 1)
                if i < M:
                    zt_next = _Mat(
                        zts.tile(
                            [128, 2, K], F32R, tag="zt", bufs=zt_pool_bufs, name=f"zt{i + 1}"
                        )
                    )
                    _product(nc, psz, zt_next, a1, zt_prev)
                    zt_prev = zt_next

    nc.compile()
    return nc


_cached_nc = None
_last_results = None


def kernel(x, W, T=None):
    global _cached_nc, _last_results
    if _cached_nc is None:
        _cached_nc = _build_program()
    nc = _cached_nc

    x2 = np.ascontiguousarray(np.asarray(x, dtype=np.float32).reshape(B_FULL, K))
    w2 = np.ascontiguousarray(np.asarray(W, dtype=np.float32))
    in_maps = [
        {"x": x2[i * B_SH : (i + 1) * B_SH], "w": w2} for i in range(N_CORES)
    ]
    res = run_bass_kernel_spmd(
        nc,
        in_maps,
        core_ids=list(range(N_CORES)),
        trace=bool(os.environ.get("BASS_TRACE")),
    )
    _last_results = res
    y = np.concatenate([res.results[i]["y"] for i in range(N_CORES)], axis=0)
    return y
